# revision 3
# baseline (speedup 1.0000x reference)
"""Trainium2 Bass kernel for single-head attention (nn_Attention_31344671326347).

Problem: B=4, S=2048, E=D=1024, fp32.
    q = x @ Wq.T + bq ; k = x @ Wk.T + bk ; v = x @ Wv.T + bv
    out = softmax(q k^T / sqrt(D)) @ v

Sharding: 8 cores = (4 batches) x (2 query-halves). Each core computes K/V
for its batch's full sequence (duplicated across the pair) and attention for
its 1024-row query half. No collectives.

Layout trick: all matmul contractions run with the contracted dim on SBUF
partitions. Host ships x^T and W^T so q^T [d,s], k^T [d,t] and v [t,d] come
straight out of the PE with zero on-device transposes; softmax runs over the
partition dim via exp (ScalarE) + a ones-matmul denominator (PE).

Matmuls use float32r (TF32-like, full PE rate at free-dim>=256).
fp32r ISA constraints honored: M=128 output partitions, even moving free dim,
contiguous 8B-aligned PSUM dst.
"""

import numpy as np

import concourse.bass as bass
import concourse.mybir as mybir
import concourse.tile as tile
from concourse import bacc
from concourse.bass_utils import run_bass_kernel_spmd

B, S, E, D = 4, 2048, 1024, 1024
SQ = S // 2          # query rows per core
P = 128
EO = E // P          # 8 contraction chunks
DO = D // P          # 8 d chunks
TC = S // P          # 16 key/t chunks
SB = SQ // 512       # 2 big s chunks
F32 = mybir.dt.float32
F32R = mybir.dt.float32r

N_CORES = 8
TRACE = False        # test.py flips this for profiling
LAST_RESULT = None   # BassKernelResults of the most recent run

_NC = None


def _build():
    nc = bacc.Bacc("TRN2", target_bir_lowering=False, debug=False,
                   num_devices=N_CORES)

    xT = nc.dram_tensor("xT", [E, S], F32R, kind="ExternalInput")
    xTq = nc.dram_tensor("xTq", [E, SQ], F32R, kind="ExternalInput")
    wqT = nc.dram_tensor("wqT", [E, D], F32R, kind="ExternalInput")
    wkT = nc.dram_tensor("wkT", [E, D], F32R, kind="ExternalInput")
    wvT = nc.dram_tensor("wvT", [E, D], F32R, kind="ExternalInput")
    bq = nc.dram_tensor("bq", [P, DO], F32, kind="ExternalInput")
    bk = nc.dram_tensor("bk", [P, DO], F32, kind="ExternalInput")
    bv = nc.dram_tensor("bv", [P, D], F32, kind="ExternalInput")
    ones_d = nc.dram_tensor("ones", [P, 2], F32R, kind="ExternalInput")
    out = nc.dram_tensor("out", [SQ, D], F32, kind="ExternalOutput")

    xT_r = xT.rearrange("(eo p) s -> p eo s", p=P)
    xTq_r = xTq.rearrange("(eo p) s -> p eo s", p=P)
    wqT_r = wqT.rearrange("(eo p) d -> p eo d", p=P)
    wkT_r = wkT.rearrange("(eo p) d -> p eo d", p=P)
    wvT_r = wvT.rearrange("(eo p) d -> p eo d", p=P)

    Ident = mybir.ActivationFunctionType.Identity
    Exp = mybir.ActivationFunctionType.Exp

    with tile.TileContext(nc) as tc:
        with (
            tc.tile_pool(name="res", bufs=1) as res,
            tc.tile_pool(name="small", bufs=1) as small,
            tc.tile_pool(name="dram", bufs=1, space="DRAM") as dram_pool,
        ):
            qT_dram = dram_pool.tile([D, SQ], F32R)
            qT_dr = qT_dram.rearrange("(do p) s -> p do s", p=P)
            bq_t = small.tile([P, DO], F32, tag="bq")
            bk_t = small.tile([P, DO], F32, tag="bk")
            bv_t = small.tile([P, D], F32, tag="bv")
            ones_t = small.tile([P, 2], F32R, tag="ones")
            nc.sync.dma_start(bq_t[:], bq[:])
            nc.sync.dma_start(bk_t[:], bk[:])
            nc.sync.dma_start(bv_t[:], bv[:])
            nc.sync.dma_start(ones_t[:], ones_d[:])

            # ---- Q projection: qT[d, s] = Wq @ x^T (+ bq per-partition) ----
            with (
                tc.tile_pool(name="wq", bufs=1) as wq_pool,
                tc.tile_pool(name="xs", bufs=2) as xs_pool,
                tc.tile_pool(name="psA", bufs=4, space="PSUM") as psA,
            ):
                wq_t = wq_pool.tile([P, EO, D], F32R, tag="wq")
                nc.sync.dma_start(wq_t[:], wqT_r[:])
                for sb in range(SB):
                    xq = xs_pool.tile([P, EO, 512], F32R, tag="xs")
                    nc.sync.dma_start(xq[:], xTq_r[:, :, sb * 512:(sb + 1) * 512])
                    for do in range(DO):
                        ps = psA.tile([P, 512], F32, tag="ps")
                        for eo in range(EO):
                            nc.tensor.matmul(
                                ps[:], wq_t[:, eo, do * P:(do + 1) * P],
                                xq[:, eo, :],
                                start=(eo == 0), stop=(eo == EO - 1),
                            )
                        qs_t = xs_pool.tile([P, 512], F32R, tag="qs")
                        nc.scalar.activation(
                            qs_t[:], ps[:], Ident, bias=bq_t[:, do:do + 1])
                        nc.sync.dma_start(
                            qT_dr[:, do, sb * 512:(sb + 1) * 512], qs_t[:])

            # ---- K projection: kT[d, t] = Wk @ x^T (+ bk per-partition) ----
            with (
                tc.tile_pool(name="wk", bufs=1) as wk_pool,
                tc.tile_pool(name="xs2", bufs=2) as xs2_pool,
                tc.tile_pool(name="psB", bufs=4, space="PSUM") as psB,
            ):
                kT_t = res.tile([P, DO, S], F32R, tag="kT")
                wk_t = wk_pool.tile([P, EO, D], F32R, tag="wk")
                nc.sync.dma_start(wk_t[:], wkT_r[:])
                for tb in range(S // 512):
                    xk = xs2_pool.tile([P, EO, 512], F32R, tag="xs2")
                    nc.sync.dma_start(xk[:], xT_r[:, :, tb * 512:(tb + 1) * 512])
                    for do in range(DO):
                        ps = psB.tile([P, 512], F32, tag="ps")
                        for eo in range(EO):
                            nc.tensor.matmul(
                                ps[:], wk_t[:, eo, do * P:(do + 1) * P],
                                xk[:, eo, :],
                                start=(eo == 0), stop=(eo == EO - 1),
                            )
                        nc.scalar.activation(
                            kT_t[:, do, tb * 512:(tb + 1) * 512], ps[:],
                            Ident, bias=bk_t[:, do:do + 1],
                        )

            # ---- V projection: v[t, d] = x @ Wv.T (+ bv along free dim) ----
            with (
                tc.tile_pool(name="wv", bufs=1) as wv_pool,
                tc.tile_pool(name="xs3", bufs=3) as xs3_pool,
                tc.tile_pool(name="psC", bufs=4, space="PSUM") as psC,
            ):
                v_t = res.tile([P, TC, D], F32R, tag="v")
                wv_t = wv_pool.tile([P, EO, D], F32R, tag="wv")
                nc.sync.dma_start(wv_t[:], wvT_r[:])
                for tc_i in range(TC):
                    xv = xs3_pool.tile([P, EO, P], F32R, tag="xs3")
                    nc.sync.dma_start(xv[:], xT_r[:, :, tc_i * P:(tc_i + 1) * P])
                    for dh in range(2):
                        ps = psC.tile([P, 512], F32, tag="ps")
                        for eo in range(EO):
                            nc.tensor.matmul(
                                ps[:], xv[:, eo, :],
                                wv_t[:, eo, dh * 512:(dh + 1) * 512],
                                start=(eo == 0), stop=(eo == EO - 1),
                            )
                        nc.vector.tensor_add(
                            ps[:], ps[:], bv_t[:, dh * 512:(dh + 1) * 512])
                        nc.scalar.activation(
                            v_t[:, tc_i, dh * 512:(dh + 1) * 512], ps[:], Ident)

            # ---- Attention ----
            inv_sqrt_d = float(1.0 / np.sqrt(D))
            with (
                tc.tile_pool(name="eT", bufs=1) as eT_pool,
                tc.tile_pool(name="qc", bufs=2) as qc_pool,
                tc.tile_pool(name="ot", bufs=3) as ot_pool,
                tc.tile_pool(name="rc", bufs=4) as rc_pool,
                tc.tile_pool(name="psS", bufs=4, space="PSUM") as psS,
                tc.tile_pool(name="psO", bufs=2, space="PSUM") as psO,
                tc.tile_pool(name="psD", bufs=2, space="PSUM") as psD,
            ):
                for sb in range(SB):
                    qc_t = qc_pool.tile([P, DO, 512], F32R, tag="qc")
                    nc.sync.dma_start(
                        qc_t[:], qT_dr[:, :, sb * 512:(sb + 1) * 512])
                    eT = eT_pool.tile([P, TC, 512], F32R, tag="eT")
                    # scoresT[t, s] then eT = exp(scoresT / sqrt(D))
                    for tc_i in range(TC):
                        ps = psS.tile([P, 512], F32, tag="ps")
                        for do in range(DO):
                            nc.tensor.matmul(
                                ps[:], kT_t[:, do, tc_i * P:(tc_i + 1) * P],
                                qc_t[:, do, :],
                                start=(do == 0), stop=(do == DO - 1),
                            )
                        nc.scalar.activation(
                            eT[:, tc_i, :], ps[:], Exp, scale=inv_sqrt_d)

                    # PV + denominator, 128 query rows at a time
                    for ss in range(4):
                        s_lo = ss * P
                        pd = psD.tile([P, 2], F32, tag="pd")
                        for tc_i in range(TC):
                            nc.tensor.matmul(
                                pd[:], eT[:, tc_i, s_lo:s_lo + P], ones_t[:],
                                start=(tc_i == 0), stop=(tc_i == TC - 1),
                            )
                        recip = rc_pool.tile([P, 1], F32, tag="recip")
                        nc.vector.reciprocal(recip[:], pd[:, 0:1])
                        for dh in range(2):
                            po = psO.tile([P, 512], F32, tag="po")
                            for tc_i in range(TC):
                                nc.tensor.matmul(
                                    po[:], eT[:, tc_i, s_lo:s_lo + P],
                                    v_t[:, tc_i, dh * 512:(dh + 1) * 512],
                                    start=(tc_i == 0), stop=(tc_i == TC - 1),
                                )
                            o_t = ot_pool.tile([P, 512], F32, tag="ot")
                            nc.vector.tensor_scalar_mul(o_t[:], po[:], recip[:])
                            nc.sync.dma_start(
                                out[sb * 512 + s_lo: sb * 512 + s_lo + P,
                                    dh * 512:(dh + 1) * 512],
                                o_t[:],
                            )

    nc.compile()
    return nc


def _get_nc():
    global _NC
    if _NC is None:
        _NC = _build()
    return _NC


def kernel(x, Wq, bq, Wk, bk, Wv, bv):
    global LAST_RESULT
    x = np.ascontiguousarray(np.asarray(x, dtype=np.float32))
    Wq = np.asarray(Wq, dtype=np.float32)
    Wk = np.asarray(Wk, dtype=np.float32)
    Wv = np.asarray(Wv, dtype=np.float32)
    bq = np.asarray(bq, dtype=np.float32)
    bk = np.asarray(bk, dtype=np.float32)
    bv = np.asarray(bv, dtype=np.float32)

    wqT = np.ascontiguousarray(Wq.T)
    wkT = np.ascontiguousarray(Wk.T)
    wvT = np.ascontiguousarray(Wv.T)
    bq_r = np.ascontiguousarray(bq.reshape(DO, P).T)
    bk_r = np.ascontiguousarray(bk.reshape(DO, P).T)
    bv_r = np.ascontiguousarray(np.broadcast_to(bv, (P, D)))
    ones = np.ones((P, 2), dtype=np.float32)

    xT_b = [np.ascontiguousarray(x[b].T) for b in range(B)]

    in_maps = []
    for c in range(N_CORES):
        b, h = divmod(c, 2)
        in_maps.append({
            "xT": xT_b[b],
            "xTq": np.ascontiguousarray(xT_b[b][:, h * SQ:(h + 1) * SQ]),
            "wqT": wqT, "wkT": wkT, "wvT": wvT,
            "bq": bq_r, "bk": bk_r, "bv": bv_r,
            "ones": ones,
        })

    nc = _get_nc()
    res = run_bass_kernel_spmd(nc, in_maps, list(range(N_CORES)), trace=TRACE)
    LAST_RESULT = res

    out = np.empty((B, S, D), dtype=np.float32)
    for c in range(N_CORES):
        b, h = divmod(c, 2)
        out[b, h * SQ:(h + 1) * SQ, :] = res.results[c]["out"]
    return out
